# revision 20
# baseline (speedup 1.0000x reference)
"""LoRA MultiheadAttention on 8 NeuronCores (Bass/Tile), v3.

Sharding: 32 (batch, head) attention slices -> 4 heads x 1 batch per core.
Cores 0-3 take batch 0, cores 4-7 batch 1; core c handles heads
(c%4)*4 .. (c%4)*4+3, i.e. a contiguous 256-wide slice of the head dims.

Structure: one ACT-saturated attention stream with all projection work
interleaved as PE filler.

  - Units are (head-pair, ti-quarter).  Per tj tile the two heads' score
    matmuls (K=64) are emitted back-to-back with lhsT/rhs at partition
    bases 0 and 64, so they run CONCURRENTLY in different PE row strips
    (tile_position (0,0)/(64,0)) -- 2x on the score cost.
  - Both heads' scores land in one [128,1024] 2-bank PSUM tile, so each
    exp is a single [128,1024] ACT instruction.
  - Phases A (LoRA down-proj), B (Q/K proj), C (V proj) and the out-proj
    are pull-scheduled filler between attention steps.
  - Inputs arrive as a handful of large multi-k-tile DMAs (the Sync
    engine's ~0.65us per dma_start issue was gating the ramp); xa is
    split into column halves so the first units' data lands first.
  - Biases ride the K=17 LoRA matmuls (ones row in ak/av).  V gets its
    softmax-denominator ones-column the same way.
  - po is evacuated PSUM->SBUF right after each unit; denominator
    broadcast via DRAM round-trip (PE K=1 broadcast matmul for the last
    unit to cut the tail); one [128,512] reciprocal + one fused multiply
    per unit.
  - Output stored bf16 (host accumulates fp32).
"""

import sys

sys.path.insert(0, "/opt/trn_rl_repo")

import math
from contextlib import ExitStack

import ml_dtypes
import numpy as np

import concourse.bass as bass
import concourse.tile as tile
from concourse import mybir
from concourse.bass_utils import run_bass_kernel_spmd

BF16 = ml_dtypes.bfloat16
F32 = mybir.dt.float32
BF = mybir.dt.bfloat16
FP8 = mybir.dt.float8e4

T = 2048
D = 1024
H = 16
HD = 64
R = 16
BSZ = 2
SCALE = 16.0
NCORES = 8
HPC = 4  # heads per core
CD = HPC * HD  # 256 head dims per core
VW = HD + 1  # V block width per head (ones column appended)
P = 128
NKT = D // P  # 8 k-tiles
NTT = T // P  # 16 tj tiles
QW = 512  # ti quarter width
NQ = T // QW  # 4 quarters
WVW = HPC * VW  # 260


def build_nc():
    nc = bass.Bass()
    xa = nc.dram_tensor("xa", [D, T], BF, kind="ExternalInput")
    wqk = nc.dram_tensor("wqk", [D, 2 * CD], BF, kind="ExternalInput")
    wv = nc.dram_tensor("wv", [D, WVW], BF, kind="ExternalInput")
    abm = nc.dram_tensor("abm", [D, 3 * R], BF, kind="ExternalInput")
    qkb = nc.dram_tensor("qkb", [R + 1, 2 * CD], BF, kind="ExternalInput")
    vbb = nc.dram_tensor("vbb", [R + 1, WVW], BF, kind="ExternalInput")
    wo = nc.dram_tensor("wo", [CD, D], BF, kind="ExternalInput")
    out = nc.dram_tensor("out", [T, D], BF, kind="ExternalOutput")

    with tile.TileContext(nc) as tc, ExitStack() as ctx:
        singles = ctx.enter_context(tc.tile_pool(name="singles", bufs=1))

        # merged per-k-tile input tiles: [128, NKT*width], k-tile kt at
        # columns [kt*width, (kt+1)*width)
        xa_all = singles.tile([P, NKT * T], BF, tag="xa_all")
        wqk_all = singles.tile([P, NKT * 2 * CD], BF, tag="wqk_all")
        wv_all = singles.tile([P, NKT * WVW], BF, tag="wv_all")
        ab_all = singles.tile([P, NKT * 3 * R], BF, tag="ab_all")
        qkb_t = singles.tile([R + 1, 2 * CD], BF, tag="qkb")
        vbb_t = singles.tile([R + 1, WVW], BF, tag="vbb")
        wo_t = [singles.tile([P, D], BF, name=f"wo{i}", tag=f"wo{i}") for i in range(2)]
        ones64 = singles.tile([1, HD], F32, tag="ones64")
        nc.vector.memset(ones64, 1.0)

        def xs(kt, lo, hi):
            return xa_all[:, kt * T + lo : kt * T + hi]

        def wqks(kt, lo, hi):
            return wqk_all[:, kt * 2 * CD + lo : kt * 2 * CD + hi]

        def wvs(kt):
            return wv_all[:, kt * WVW : (kt + 1) * WVW]

        def abs_(kt):
            return ab_all[:, kt * 3 * R : (kt + 1) * 3 * R]

        # merged input DMAs: one start per tensor (xa in column halves),
        # ordered so the prologue's data lands first.
        def load_ktiled(dst_tile, dst_w, src, src_w, lo, hi):
            w = hi - lo
            dst = bass.AP(
                tensor=dst_tile.tensor,
                offset=dst_tile.offset + lo,
                ap=[list(dst_tile.ap[0]), [dst_w, NKT], [1, w]],
            )
            srcap = bass.AP(
                tensor=src[0:1, 0:1].tensor,
                offset=lo,
                ap=[[src_w, P], [P * src_w, NKT], [1, w]],
            )
            nc.sync.dma_start(out=dst, in_=srcap)

        def load_wqk_mgroup(g):
            # columns of m-tiles m=g and m=2+g for every k-tile (3-dim APs)
            for base in (g * P, CD + g * P):
                dst = bass.AP(
                    tensor=wqk_all.tensor,
                    offset=wqk_all.offset + base,
                    ap=[list(wqk_all.ap[0]), [2 * CD, NKT], [1, P]],
                )
                srcap = bass.AP(
                    tensor=wqk[0:1, 0:1].tensor,
                    offset=base,
                    ap=[[2 * CD, P], [P * 2 * CD, NKT], [1, P]],
                )
                nc.sync.dma_start(out=dst, in_=srcap)

        load_ktiled(xa_all, T, xa, T, 0, T // 4)
        load_ktiled(ab_all, 3 * R, abm, 3 * R, 0, 3 * R)
        nc.sync.dma_start(out=qkb_t, in_=qkb[:, :])
        load_wqk_mgroup(0)
        load_ktiled(xa_all, T, xa, T, T // 4, T // 2)
        load_ktiled(wv_all, WVW, wv, WVW, 0, WVW)
        load_wqk_mgroup(1)
        load_ktiled(xa_all, T, xa, T, T // 2, 3 * T // 4)
        load_ktiled(xa_all, T, xa, T, 3 * T // 4, T)
        nc.sync.dma_start(out=vbb_t, in_=vbb[:, :])
        for i in range(2):
            nc.sync.dma_start(out=wo_t[i], in_=wo[i * P : (i + 1) * P, :])

        # ak/av: LoRA down-projections with a trailing ones row (bias lane).
        # DVE accesses must start at partition 0/32/64/96: memset the whole
        # tile to 1.0 (covers the ones row); Phase A overwrites rows 0..15.
        ak_sb = singles.tile([R + 1, T], BF, tag="ak")
        av_sb = singles.tile([R + 1, T], BF, tag="av")
        nc.vector.memset(ak_sb, 1.0)
        nc.vector.memset(av_sb, 1.0)

        qk_sb = [singles.tile([P, T], BF, name=f"qk{i}", tag=f"qk{i}") for i in range(4)]
        VST = 272  # 16-aligned stride of one tj tile inside a v2 pair tile
        v2_sb = [singles.tile([P, 2 * VST], FP8, name=f"v2_{i}", tag=f"v2_{i}") for i in range(NTT // 2)]
        oT_sb = [singles.tile([P, T], BF, name=f"oT{i}", tag=f"oT{i}") for i in range(2)]

        # attention pools (long-lived, below proj on the pool stack)
        pS = ctx.enter_context(tc.tile_pool(name="pS", bufs=2, space="PSUM"))
        pO = ctx.enter_context(tc.tile_pool(name="pO", bufs=1, space="PSUM"))
        pP = ctx.enter_context(tc.tile_pool(name="pP", bufs=3))
        pN = ctx.enter_context(tc.tile_pool(name="pN", bufs=1))
        pPo = ctx.enter_context(tc.tile_pool(name="pPo", bufs=2))
        pD = ctx.enter_context(tc.tile_pool(name="pD", bufs=2, space="DRAM"))
        pOut = ctx.enter_context(tc.tile_pool(name="pOut", bufs=2))

        # proj pool: one rotating PSUM tag shared by phases A, B and C;
        # allocated last (stack top) and released once all A/B/C fillers
        # have been emitted so its banks recycle into the out-proj pool.
        proj = tc.alloc_tile_pool(name="proj", bufs=2, space="PSUM")

        def emit_a(ch):
            # [ak; av] chunk = [k_a; v_a] @ X   (rows 0..15 / 32..47)
            cs = slice(ch * 512, (ch + 1) * 512)
            pa = proj.tile([3 * R, 512], F32, tag="proj", name=f"pa{ch}")
            for kt in range(NKT):
                nc.tensor.matmul(
                    pa,
                    lhsT=abs_(kt),
                    rhs=xs(kt, ch * 512, (ch + 1) * 512),
                    start=(kt == 0),
                    stop=(kt == NKT - 1),
                )
            nc.vector.tensor_copy(ak_sb[0:R, cs], pa[0:R, :])
            nc.vector.tensor_copy(av_sb[0:R, cs], pa[2 * R : 3 * R, :])

        def emit_b(m, ch):
            # Q^T / K^T m-tile (m=0,1 -> Q pair m; m=2,3 -> K pair m-2), ti/tj
            # chunk ch.  8 k-tiles + 1 K=17 LoRA/bias matmul.
            cs = slice(ch * 512, (ch + 1) * 512)
            pq = proj.tile([P, 512], F32, tag="proj", name=f"pq_{m}_{ch}")
            for kt in range(NKT):
                nc.tensor.matmul(
                    pq,
                    lhsT=wqks(kt, m * P, (m + 1) * P),
                    rhs=xs(kt, ch * 512, (ch + 1) * 512),
                    start=(kt == 0),
                    stop=False,
                )
            nc.tensor.matmul(
                pq,
                lhsT=qkb_t[:, m * P : (m + 1) * P],
                rhs=ak_sb[:, cs],
                start=False,
                stop=True,
            )
            nc.vector.tensor_copy(qk_sb[m][:, cs], pq)

        def emit_c(mt):
            # V m-tile mt in natural (T, 4*65) layout; LoRA + bias + ones-col
            # via the K=17 matmul.
            pv = proj.tile([P, WVW], F32, tag="proj", name=f"pv_{mt}")
            for kt in range(NKT):
                nc.tensor.matmul(
                    pv,
                    lhsT=xs(kt, mt * P, (mt + 1) * P),
                    rhs=wvs(kt),
                    start=(kt == 0),
                    stop=False,
                )
            nc.tensor.matmul(
                pv,
                lhsT=av_sb[:, mt * P : (mt + 1) * P],
                rhs=vbb_t,
                start=False,
                stop=True,
            )
            nc.vector.tensor_copy(
                v2_sb[mt // 2][:, (mt % 2) * VST : (mt % 2) * VST + WVW], pv
            )

        # out-proj PSUM pool opened lazily (after proj retires) to stay
        # within 8 banks
        pE_box = {}

        def get_pE():
            if "pE" not in pE_box:
                pE_box["pE"] = ctx.enter_context(
                    tc.tile_pool(name="pE", bufs=2, space="PSUM")
                )
            return pE_box["pE"]

        def emit_outproj_mt(mt):
            pE = get_pE()
            ms = slice(mt * P, (mt + 1) * P)
            ob = pOut.tile([P, D], BF, tag="ob", name=f"ob_{mt}")
            for chh in range(2):
                cs = slice(chh * 512, (chh + 1) * 512)
                po2 = pE.tile([P, 512], F32, tag="po2", name=f"po2_{mt}_{chh}")
                for kt2 in range(2):
                    nc.tensor.matmul(
                        po2,
                        lhsT=oT_sb[kt2][:, ms],
                        rhs=wo_t[kt2][:, cs],
                        start=(kt2 == 0),
                        stop=(kt2 == 1),
                    )
                nc.vector.tensor_copy(ob[:, cs], po2)
                nc.sync.dma_start(out=out[ms, cs], in_=ob[:, cs])

        fillers = {}
        for ch in range(4):
            fillers[f"a{ch}"] = (lambda ch=ch: emit_a(ch))
        for m in range(4):
            for ch in range(4):
                fillers[f"b{m}{ch}"] = (lambda m=m, ch=ch: emit_b(m, ch))
        for mt in range(NTT):
            fillers[f"c{mt}"] = (lambda mt=mt: emit_c(mt))
        for mt in range(NTT):
            fillers[f"o{mt}"] = (lambda mt=mt: emit_outproj_mt(mt))
        fillers["release"] = proj.release

        def pull(key):
            f = fillers.pop(key, None)
            if f is not None:
                f()

        # prologue: minimal work for unit (pair 0, quarter 0)
        pull("b00")
        pull("a0")
        pull("b20")

        # filler pull schedule per (pair, quarter) unit: {step: [keys]}
        unit_pulls = {
            (0, 0): {
                0: ["c0"], 1: ["a1", "c1"], 2: ["b21", "c2"], 3: ["c3", "c4"],
                4: ["a2", "c5"], 5: ["b22", "c6"], 6: ["c7"], 7: ["c8"],
                8: ["a3", "c9"], 9: ["b23", "c10"], 10: ["c11"], 11: ["c12"],
                12: ["c13"], 13: ["c14"], 14: ["c15", "b01"],
            },
            (0, 1): {0: ["b30"], 8: ["b02"], 12: ["b31"]},
            (0, 2): {0: ["b32"], 8: ["b03"]},
            (0, 3): {0: ["b33"], 12: ["b10"]},
            (1, 0): {4: ["b11"], 12: ["b12"]},
            (1, 1): {0: ["b13"], 1: ["release"], 2: ["o0"], 6: ["o1"], 10: ["o2"], 14: ["o3"]},
            (1, 2): {2: ["o4"], 6: ["o5"], 10: ["o6"], 14: ["o7"]},
            (1, 3): {2: ["o8"], 6: ["o9"], 10: ["o10"], 14: ["o11"]},
        }

        def emit_norm(p, q, po_sb, den_e, den_o, pe_bcast=False):
            # broadcast the two 512-wide denominators across 64 partitions
            # each, then one reciprocal and one fused [128,512] multiply for
            # both heads.  DRAM round-trip normally; K=1 PE broadcast matmul
            # for the last unit (cuts two DMA hops from the tail).
            qs = slice(q * QW, (q + 1) * QW)
            if pe_bcast:
                pE = get_pE()
                pbc = pE.tile([P, QW], F32, tag="po2", name=f"pbc_{p}_{q}")
                nc.tensor.matmul(
                    pbc[0:HD, :], lhsT=ones64, rhs=den_e, start=True, stop=True
                )
                nc.tensor.matmul(
                    pbc[HD:P, :], lhsT=ones64, rhs=den_o, start=True, stop=True
                )
                d128 = pbc
            else:
                dre = pD.tile([1, QW], F32, tag="dre", name=f"dre_{p}_{q}")
                dro = pD.tile([1, QW], F32, tag="dro", name=f"dro_{p}_{q}")
                nc.sync.dma_start(out=dre, in_=den_e)
                nc.sync.dma_start(out=dro, in_=den_o)
                d128 = pN.tile([P, QW], F32, tag="d128", name=f"d128_{p}_{q}")
                nc.sync.dma_start(
                    out=d128[0:HD, :],
                    in_=bass.AP(
                        tensor=dre.tensor, offset=dre.offset, ap=[[0, HD], [1, QW]]
                    ),
                )
                nc.sync.dma_start(
                    out=d128[HD:P, :],
                    in_=bass.AP(
                        tensor=dro.tensor, offset=dro.offset, ap=[[0, HD], [1, QW]]
                    ),
                )
            rec = pN.tile([P, QW], F32, tag="rec", name=f"rec_{p}_{q}")
            for cc in range(4):
                ccs = slice(cc * P, (cc + 1) * P)
                ocs = slice(q * QW + cc * P, q * QW + (cc + 1) * P)
                nc.vector.reciprocal(out=rec[:, ccs], in_=d128[:, ccs])
                nc.vector.tensor_mul(oT_sb[p][:, ocs], po_sb[:, ccs], rec[:, ccs])

        norm_pending = None
        for p in range(2):
            qT = qk_sb[p]
            kT = qk_sb[2 + p]
            for q in range(NQ):
                qs = slice(q * QW, (q + 1) * QW)
                pulls = unit_pulls[(p, q)]
                for k in pulls.get(-1, []):
                    pull(k)
                poe = pO.tile([VW, QW], F32, tag="poe", name=f"poe_{p}_{q}")
                poo = pO.tile([VW, QW], F32, tag="poo", name=f"poo_{p}_{q}")
                pts = {}

                def emit_pv(tjd, poe=poe, poo=poo, p=p):
                    # fp8 DoubleRow: one matmul contracts both tj tiles of a
                    # pair (virtual K=256); lhsT/rhs carry the pair in dim 1.
                    ptd = pts.pop(tjd)
                    v2 = v2_sb[tjd]
                    for hloc, po_ in ((2 * p, poe), (2 * p + 1, poo)):
                        lhsT = bass.AP(
                            tensor=v2.tensor,
                            offset=v2.offset + hloc * VW,
                            ap=[list(v2.ap[0]), [VST, 2], [1, VW]],
                        )
                        rhs = bass.AP(
                            tensor=ptd.tensor,
                            offset=ptd.offset + (hloc - 2 * p) * QW,
                            ap=[list(ptd.ap[0]), [2 * QW, 2], [1, QW]],
                        )
                        nc.tensor.matmul(
                            po_,
                            lhsT=lhsT,
                            rhs=rhs,
                            perf_mode=mybir.MatmulPerfMode.DoubleRow,
                            start=(tjd == 0),
                            stop=(tjd == NTT // 2 - 1),
                        )

                for tj in range(NTT):
                    tjs = slice(tj * P, (tj + 1) * P)
                    sp = pS.tile([P, 2 * QW], F32, tag="sp", name=f"sp_{p}_{q}_{tj}")
                    nc.tensor.matmul(
                        sp[:, 0:QW],
                        lhsT=kT[0:HD, tjs],
                        rhs=qT[0:HD, qs],
                        start=True,
                        stop=True,
                    )
                    nc.tensor.matmul(
                        sp[:, QW : 2 * QW],
                        lhsT=kT[HD:P, tjs],
                        rhs=qT[HD:P, qs],
                        start=True,
                        stop=True,
                    )
                    if tj % 2 == 0:
                        ptd = pP.tile(
                            [P, 4 * QW], FP8, tag="pt", name=f"pt_{p}_{q}_{tj}"
                        )
                        pts[tj // 2] = ptd
                    else:
                        ptd = pts[tj // 2]
                    nc.scalar.activation(
                        ptd[:, (tj % 2) * 2 * QW : (tj % 2 + 1) * 2 * QW],
                        sp,
                        mybir.ActivationFunctionType.Exp,
                    )
                    if tj >= 2 and tj % 2 == 0:
                        emit_pv(tj // 2 - 1)
                    for k in pulls.get(tj, []):
                        pull(k)
                    # emit previous unit's norm once this unit is underway
                    if tj == 1 and norm_pending is not None:
                        emit_norm(*norm_pending)
                        norm_pending = None
                emit_pv(NTT // 2 - 1)

                # evacuate po to SBUF so the PSUM banks free for the next
                # unit: both heads' O^T into one [128,512] pair tile, the two
                # denominator rows into [1,512] f32 tiles.
                po_sb = pPo.tile([P, QW], F32, tag="po_sb", name=f"posb_{p}_{q}")
                den_e = pPo.tile([1, QW], F32, tag="den_e", name=f"dene_{p}_{q}")
                den_o = pPo.tile([1, QW], F32, tag="den_o", name=f"deno_{p}_{q}")
                nc.vector.tensor_copy(den_e, poe[HD : HD + 1, :])
                nc.vector.tensor_copy(den_o, poo[HD : HD + 1, :])
                nc.vector.tensor_copy(po_sb[0:HD, :], poe[0:HD, :])
                nc.vector.tensor_copy(po_sb[HD:P, :], poo[0:HD, :])
                norm_pending = (p, q, po_sb, den_e, den_o)

        emit_norm(*norm_pending, pe_bcast=True)
        for mt in range(12, 16):
            pull(f"o{mt}")

    # bass.Bass's finalize skips Bacc's wait-splitting passes; walrus allows
    # at most 1 sync wait per instruction (2 for event semaphores), so run
    # just those two passes here.
    import bass_rust as _bass_rust

    _bass_rust.move_matmul_waits_to_ldweights(nc.m)
    _bass_rust.generate_event_semaphores(nc)
    return nc


def prepare_in_maps(inputs):
    q = np.asarray(inputs["query"], np.float32)
    ipw = np.asarray(inputs["in_proj_weight"], np.float32)
    ipb = np.asarray(inputs["in_proj_bias"], np.float32)
    out_w = np.asarray(inputs["out_w"], np.float32)
    k_a = np.asarray(inputs["k_a"], np.float32)
    k_b = np.asarray(inputs["k_b"], np.float32)
    v_a = np.asarray(inputs["v_a"], np.float32)
    v_b = np.asarray(inputs["v_b"], np.float32)
    qscale = 1.0 / math.sqrt(HD)
    sl = SCALE / R

    in_maps = []
    for c in range(NCORES):
        bb = c // 4
        s = (c % 4) * CD
        e = s + CD
        X = q[:, bb, :]

        xa = np.ascontiguousarray(X.T)

        wqk = np.empty((D, 2 * CD), np.float32)
        wqk[:, :CD] = ipw[s:e].T * qscale
        wqk[:, CD:] = ipw[D + s : D + e].T

        qkb = np.zeros((R + 1, 2 * CD), np.float32)
        qkb[:R, CD:] = k_b[:, s:e] * sl
        qkb[R, :CD] = ipb[s:e] * qscale
        qkb[R, CD:] = ipb[D + s : D + e]

        wv = np.zeros((D, WVW), np.float32)
        vbb = np.zeros((R + 1, WVW), np.float32)
        for j in range(HPC):
            wv[:, j * VW : j * VW + HD] = ipw[2 * D + s + j * HD : 2 * D + s + (j + 1) * HD].T
            vbb[:R, j * VW : j * VW + HD] = v_b[:, s + j * HD : s + (j + 1) * HD] * sl
            vbb[R, j * VW : j * VW + HD] = ipb[2 * D + s + j * HD : 2 * D + s + (j + 1) * HD]
            vbb[R, j * VW + HD] = 1.0

        abm = np.zeros((D, 3 * R), np.float32)
        abm[:, :R] = k_a.T
        abm[:, 2 * R :] = v_a.T

        wo = out_w[:, s:e].T

        in_maps.append(
            {
                "xa": xa.astype(BF16),
                "wqk": wqk.astype(BF16),
                "wv": wv.astype(BF16),
                "abm": abm.astype(BF16),
                "qkb": qkb.astype(BF16),
                "vbb": vbb.astype(BF16),
                "wo": wo.astype(BF16),
            }
        )
    return in_maps


def assemble_output(inputs, results):
    out_b = np.asarray(inputs["out_b"], np.float32)
    out = np.zeros((T, BSZ, D), np.float32)
    for c in range(NCORES):
        out[:, c // 4, :] += np.asarray(results[c]["out"], np.float32)
    out += out_b[None, None, :]
    return out


def kernel(**inputs):
    nc = build_nc()
    in_maps = prepare_in_maps(inputs)
    res = run_bass_kernel_spmd(nc, in_maps, core_ids=list(range(NCORES)))
    return assemble_output(inputs, res.results)


# revision 21
# speedup vs baseline: 1.1532x; 1.1532x over previous
"""LoRA MultiheadAttention on 8 NeuronCores (Bass/Tile), v3.

Sharding: 32 (batch, head) attention slices -> 4 heads x 1 batch per core.
Cores 0-3 take batch 0, cores 4-7 batch 1; core c handles heads
(c%4)*4 .. (c%4)*4+3, i.e. a contiguous 256-wide slice of the head dims.

Structure: one ACT-saturated attention stream with all projection work
interleaved as PE filler.

  - Units are (head-pair, ti-quarter).  Per tj tile the two heads' score
    matmuls (K=64) are emitted back-to-back with lhsT/rhs at partition
    bases 0 and 64, so they run CONCURRENTLY in different PE row strips
    (tile_position (0,0)/(64,0)) -- 2x on the score cost.
  - Both heads' scores land in one [128,1024] 2-bank PSUM tile, so each
    exp is a single [128,1024] ACT instruction.
  - Phases A (LoRA down-proj), B (Q/K proj), C (V proj) and the out-proj
    are pull-scheduled filler between attention steps.
  - Inputs arrive as a handful of large multi-k-tile DMAs (the Sync
    engine's ~0.65us per dma_start issue was gating the ramp); xa is
    split into column halves so the first units' data lands first.
  - Biases ride the K=17 LoRA matmuls (ones row in ak/av).  V gets its
    softmax-denominator ones-column the same way.
  - po is evacuated PSUM->SBUF right after each unit; denominator
    broadcast via DRAM round-trip (PE K=1 broadcast matmul for the last
    unit to cut the tail); one [128,512] reciprocal + one fused multiply
    per unit.
  - Output stored bf16 (host accumulates fp32).
"""

import sys

sys.path.insert(0, "/opt/trn_rl_repo")

import math
from contextlib import ExitStack

import ml_dtypes
import numpy as np

import concourse.bass as bass
import concourse.tile as tile
from concourse import mybir
from concourse.bass_utils import run_bass_kernel_spmd

BF16 = ml_dtypes.bfloat16
F32 = mybir.dt.float32
BF = mybir.dt.bfloat16
FP8 = mybir.dt.float8e4

T = 2048
D = 1024
H = 16
HD = 64
R = 16
BSZ = 2
SCALE = 16.0
NCORES = 8
HPC = 4  # heads per core
CD = HPC * HD  # 256 head dims per core
VW = HD + 1  # V block width per head (ones column appended)
P = 128
NKT = D // P  # 8 k-tiles
NTT = T // P  # 16 tj tiles
QW = 512  # ti quarter width
NQ = T // QW  # 4 quarters
WVW = HPC * VW  # 260


def build_nc():
    nc = bass.Bass()
    xa = nc.dram_tensor("xa", [D, T], BF, kind="ExternalInput")
    wqk = nc.dram_tensor("wqk", [D, 2 * CD], BF, kind="ExternalInput")
    wv = nc.dram_tensor("wv", [D, WVW], BF, kind="ExternalInput")
    abm = nc.dram_tensor("abm", [D, 3 * R], BF, kind="ExternalInput")
    qkb = nc.dram_tensor("qkb", [R + 1, 2 * CD], BF, kind="ExternalInput")
    vbb = nc.dram_tensor("vbb", [R + 1, WVW], BF, kind="ExternalInput")
    wo = nc.dram_tensor("wo", [CD, D], BF, kind="ExternalInput")
    out = nc.dram_tensor("out", [T, D], BF, kind="ExternalOutput")

    with tile.TileContext(nc) as tc, ExitStack() as ctx:
        singles = ctx.enter_context(tc.tile_pool(name="singles", bufs=1))

        # merged per-k-tile input tiles: [128, NKT*width], k-tile kt at
        # columns [kt*width, (kt+1)*width)
        xa_all = singles.tile([P, NKT * T], BF, tag="xa_all")
        wqk_all = singles.tile([P, NKT * 2 * CD], BF, tag="wqk_all")
        wv_all = singles.tile([P, NKT * WVW], BF, tag="wv_all")
        ab_all = singles.tile([P, NKT * 3 * R], BF, tag="ab_all")
        qkb_t = singles.tile([R + 1, 2 * CD], BF, tag="qkb")
        vbb_t = singles.tile([R + 1, WVW], BF, tag="vbb")
        wo_t = [singles.tile([P, D], BF, name=f"wo{i}", tag=f"wo{i}") for i in range(2)]
        ones64 = singles.tile([1, HD], F32, tag="ones64")
        nc.vector.memset(ones64, 1.0)

        def xs(kt, lo, hi):
            return xa_all[:, kt * T + lo : kt * T + hi]

        def wqks(kt, lo, hi):
            return wqk_all[:, kt * 2 * CD + lo : kt * 2 * CD + hi]

        def wvs(kt):
            return wv_all[:, kt * WVW : (kt + 1) * WVW]

        def abs_(kt):
            return ab_all[:, kt * 3 * R : (kt + 1) * 3 * R]

        # merged input DMAs: one start per tensor (xa in column halves),
        # ordered so the prologue's data lands first.
        def load_ktiled(dst_tile, dst_w, src, src_w, lo, hi):
            w = hi - lo
            dst = bass.AP(
                tensor=dst_tile.tensor,
                offset=dst_tile.offset + lo,
                ap=[list(dst_tile.ap[0]), [dst_w, NKT], [1, w]],
            )
            srcap = bass.AP(
                tensor=src[0:1, 0:1].tensor,
                offset=lo,
                ap=[[src_w, P], [P * src_w, NKT], [1, w]],
            )
            nc.sync.dma_start(out=dst, in_=srcap)

        def load_wqk_mgroup(g):
            # columns of m-tiles m=g and m=2+g for every k-tile (3-dim APs)
            for base in (g * P, CD + g * P):
                dst = bass.AP(
                    tensor=wqk_all.tensor,
                    offset=wqk_all.offset + base,
                    ap=[list(wqk_all.ap[0]), [2 * CD, NKT], [1, P]],
                )
                srcap = bass.AP(
                    tensor=wqk[0:1, 0:1].tensor,
                    offset=base,
                    ap=[[2 * CD, P], [P * 2 * CD, NKT], [1, P]],
                )
                nc.sync.dma_start(out=dst, in_=srcap)

        load_ktiled(xa_all, T, xa, T, 0, T // 4)
        load_ktiled(ab_all, 3 * R, abm, 3 * R, 0, 3 * R)
        nc.sync.dma_start(out=qkb_t, in_=qkb[:, :])
        load_wqk_mgroup(0)
        load_ktiled(xa_all, T, xa, T, T // 4, T // 2)
        load_ktiled(wv_all, WVW, wv, WVW, 0, WVW)
        load_wqk_mgroup(1)
        load_ktiled(xa_all, T, xa, T, T // 2, 3 * T // 4)
        load_ktiled(xa_all, T, xa, T, 3 * T // 4, T)
        nc.sync.dma_start(out=vbb_t, in_=vbb[:, :])
        for i in range(2):
            nc.sync.dma_start(out=wo_t[i], in_=wo[i * P : (i + 1) * P, :])

        # ak/av: LoRA down-projections with a trailing ones row (bias lane).
        # DVE accesses must start at partition 0/32/64/96: memset the whole
        # tile to 1.0 (covers the ones row); Phase A overwrites rows 0..15.
        ak_sb = singles.tile([R + 1, T], BF, tag="ak")
        av_sb = singles.tile([R + 1, T], BF, tag="av")
        nc.vector.memset(ak_sb, 1.0)
        nc.vector.memset(av_sb, 1.0)

        qk_sb = [singles.tile([P, T], BF, name=f"qk{i}", tag=f"qk{i}") for i in range(4)]
        VST = 272  # 16-aligned stride of one tj tile inside a v2 pair tile
        v2_sb = [singles.tile([P, 2 * VST], FP8, name=f"v2_{i}", tag=f"v2_{i}") for i in range(NTT // 2)]
        oT_sb = [singles.tile([P, T], BF, name=f"oT{i}", tag=f"oT{i}") for i in range(2)]

        # attention pools (long-lived, below proj on the pool stack)
        pS = ctx.enter_context(tc.tile_pool(name="pS", bufs=2, space="PSUM"))
        pO = ctx.enter_context(tc.tile_pool(name="pO", bufs=1, space="PSUM"))
        pP = ctx.enter_context(tc.tile_pool(name="pP", bufs=3))
        pN = ctx.enter_context(tc.tile_pool(name="pN", bufs=1))
        pPo = ctx.enter_context(tc.tile_pool(name="pPo", bufs=2))
        pD = ctx.enter_context(tc.tile_pool(name="pD", bufs=2, space="DRAM"))
        pOut = ctx.enter_context(tc.tile_pool(name="pOut", bufs=2))

        # proj pool: one rotating PSUM tag shared by phases A, B and C;
        # allocated last (stack top) and released once all A/B/C fillers
        # have been emitted so its banks recycle into the out-proj pool.
        proj = tc.alloc_tile_pool(name="proj", bufs=2, space="PSUM")

        def emit_a(ch):
            # [ak; av] chunk = [k_a; v_a] @ X   (rows 0..15 / 32..47)
            cs = slice(ch * 512, (ch + 1) * 512)
            pa = proj.tile([3 * R, 512], F32, tag="proj", name=f"pa{ch}")
            for kt in range(NKT):
                nc.tensor.matmul(
                    pa,
                    lhsT=abs_(kt),
                    rhs=xs(kt, ch * 512, (ch + 1) * 512),
                    start=(kt == 0),
                    stop=(kt == NKT - 1),
                )
            nc.vector.tensor_copy(ak_sb[0:R, cs], pa[0:R, :])
            nc.vector.tensor_copy(av_sb[0:R, cs], pa[2 * R : 3 * R, :])

        def emit_b(m, ch):
            # Q^T / K^T m-tile (m=0,1 -> Q pair m; m=2,3 -> K pair m-2), ti/tj
            # chunk ch.  8 k-tiles + 1 K=17 LoRA/bias matmul.
            cs = slice(ch * 512, (ch + 1) * 512)
            pq = proj.tile([P, 512], F32, tag="proj", name=f"pq_{m}_{ch}")
            for kt in range(NKT):
                nc.tensor.matmul(
                    pq,
                    lhsT=wqks(kt, m * P, (m + 1) * P),
                    rhs=xs(kt, ch * 512, (ch + 1) * 512),
                    start=(kt == 0),
                    stop=False,
                )
            nc.tensor.matmul(
                pq,
                lhsT=qkb_t[:, m * P : (m + 1) * P],
                rhs=ak_sb[:, cs],
                start=False,
                stop=True,
            )
            nc.vector.tensor_copy(qk_sb[m][:, cs], pq)

        def emit_c(mt):
            # V m-tile mt in natural (T, 4*65) layout; LoRA + bias + ones-col
            # via the K=17 matmul.
            pv = proj.tile([P, WVW], F32, tag="proj", name=f"pv_{mt}")
            for kt in range(NKT):
                nc.tensor.matmul(
                    pv,
                    lhsT=xs(kt, mt * P, (mt + 1) * P),
                    rhs=wvs(kt),
                    start=(kt == 0),
                    stop=False,
                )
            nc.tensor.matmul(
                pv,
                lhsT=av_sb[:, mt * P : (mt + 1) * P],
                rhs=vbb_t,
                start=False,
                stop=True,
            )
            nc.vector.tensor_copy(
                v2_sb[mt // 2][:, (mt % 2) * VST : (mt % 2) * VST + WVW], pv
            )

        # out-proj PSUM pool opened lazily (after proj retires) to stay
        # within 8 banks
        pE_box = {}

        def get_pE():
            if "pE" not in pE_box:
                pE_box["pE"] = ctx.enter_context(
                    tc.tile_pool(name="pE", bufs=2, space="PSUM")
                )
            return pE_box["pE"]

        def emit_outproj_mt(mt):
            pE = get_pE()
            ms = slice(mt * P, (mt + 1) * P)
            ob = pOut.tile([P, D], BF, tag="ob", name=f"ob_{mt}")
            for chh in range(2):
                cs = slice(chh * 512, (chh + 1) * 512)
                po2 = pE.tile([P, 512], F32, tag="po2", name=f"po2_{mt}_{chh}")
                for kt2 in range(2):
                    nc.tensor.matmul(
                        po2,
                        lhsT=oT_sb[kt2][:, ms],
                        rhs=wo_t[kt2][:, cs],
                        start=(kt2 == 0),
                        stop=(kt2 == 1),
                    )
                nc.vector.tensor_copy(ob[:, cs], po2)
                nc.sync.dma_start(out=out[ms, cs], in_=ob[:, cs])

        fillers = {}
        for ch in range(4):
            fillers[f"a{ch}"] = (lambda ch=ch: emit_a(ch))
        for m in range(4):
            for ch in range(4):
                fillers[f"b{m}{ch}"] = (lambda m=m, ch=ch: emit_b(m, ch))
        for mt in range(NTT):
            fillers[f"c{mt}"] = (lambda mt=mt: emit_c(mt))
        for mt in range(NTT):
            fillers[f"o{mt}"] = (lambda mt=mt: emit_outproj_mt(mt))
        fillers["release"] = proj.release

        def pull(key):
            f = fillers.pop(key, None)
            if f is not None:
                f()

        # prologue: minimal work for unit (pair 0, quarter 0)
        pull("b00")
        pull("a0")
        pull("b20")

        # filler pull schedule per (pair, quarter) unit: {step: [keys]}
        unit_pulls = {
            (0, 0): {
                -1: ["c0", "c1"],
                0: ["a1", "c2"], 1: ["b21", "c3"], 2: ["c4"], 3: ["c5"],
                4: ["a2", "c6"], 5: ["b22", "c7"], 6: ["c8"], 7: ["c9"],
                8: ["a3", "c10"], 9: ["b23", "c11"], 10: ["c12"], 11: ["c13"],
                12: ["c14"], 13: ["c15"], 14: ["b01"],
            },
            (0, 1): {0: ["b30"], 8: ["b02"], 12: ["b31"]},
            (0, 2): {0: ["b32"], 8: ["b03"]},
            (0, 3): {0: ["b33"], 12: ["b10"]},
            (1, 0): {4: ["b11"], 12: ["b12"]},
            (1, 1): {0: ["b13"], 1: ["release"], 2: ["o0"], 6: ["o1"], 10: ["o2"], 14: ["o3"]},
            (1, 2): {2: ["o4"], 6: ["o5"], 10: ["o6"], 14: ["o7"]},
            (1, 3): {2: ["o8"], 6: ["o9"], 10: ["o10"], 14: ["o11"]},
        }

        def emit_norm(p, q, po_sb, den_e, den_o, pe_bcast=False):
            # broadcast the two 512-wide denominators across 64 partitions
            # each, then one reciprocal and one fused [128,512] multiply for
            # both heads.  DRAM round-trip normally; K=1 PE broadcast matmul
            # for the last unit (cuts two DMA hops from the tail).
            qs = slice(q * QW, (q + 1) * QW)
            if pe_bcast:
                pE = get_pE()
                pbc = pE.tile([P, QW], F32, tag="po2", name=f"pbc_{p}_{q}")
                nc.tensor.matmul(
                    pbc[0:HD, :], lhsT=ones64, rhs=den_e, start=True, stop=True
                )
                nc.tensor.matmul(
                    pbc[HD:P, :], lhsT=ones64, rhs=den_o, start=True, stop=True
                )
                d128 = pbc
            else:
                dre = pD.tile([1, QW], F32, tag="dre", name=f"dre_{p}_{q}")
                dro = pD.tile([1, QW], F32, tag="dro", name=f"dro_{p}_{q}")
                nc.sync.dma_start(out=dre, in_=den_e)
                nc.sync.dma_start(out=dro, in_=den_o)
                d128 = pN.tile([P, QW], F32, tag="d128", name=f"d128_{p}_{q}")
                nc.sync.dma_start(
                    out=d128[0:HD, :],
                    in_=bass.AP(
                        tensor=dre.tensor, offset=dre.offset, ap=[[0, HD], [1, QW]]
                    ),
                )
                nc.sync.dma_start(
                    out=d128[HD:P, :],
                    in_=bass.AP(
                        tensor=dro.tensor, offset=dro.offset, ap=[[0, HD], [1, QW]]
                    ),
                )
            rec = pN.tile([P, QW], F32, tag="rec", name=f"rec_{p}_{q}")
            for cc in range(4):
                ccs = slice(cc * P, (cc + 1) * P)
                ocs = slice(q * QW + cc * P, q * QW + (cc + 1) * P)
                nc.vector.reciprocal(out=rec[:, ccs], in_=d128[:, ccs])
                nc.vector.tensor_mul(oT_sb[p][:, ocs], po_sb[:, ccs], rec[:, ccs])

        norm_pending = None
        for p in range(2):
            qT = qk_sb[p]
            kT = qk_sb[2 + p]
            for q in range(NQ):
                qs = slice(q * QW, (q + 1) * QW)
                pulls = unit_pulls[(p, q)]
                for k in pulls.get(-1, []):
                    pull(k)
                poe = pO.tile([VW, QW], F32, tag="poe", name=f"poe_{p}_{q}")
                poo = pO.tile([VW, QW], F32, tag="poo", name=f"poo_{p}_{q}")
                pts = {}

                def emit_pv(tjd, poe=poe, poo=poo, p=p):
                    # fp8 DoubleRow: one matmul contracts both tj tiles of a
                    # pair (virtual K=256); lhsT/rhs carry the pair in dim 1.
                    ptd = pts.pop(tjd)
                    v2 = v2_sb[tjd]
                    for hloc, po_ in ((2 * p, poe), (2 * p + 1, poo)):
                        lhsT = bass.AP(
                            tensor=v2.tensor,
                            offset=v2.offset + hloc * VW,
                            ap=[list(v2.ap[0]), [VST, 2], [1, VW]],
                        )
                        rhs = bass.AP(
                            tensor=ptd.tensor,
                            offset=ptd.offset + (hloc - 2 * p) * QW,
                            ap=[list(ptd.ap[0]), [2 * QW, 2], [1, QW]],
                        )
                        nc.tensor.matmul(
                            po_,
                            lhsT=lhsT,
                            rhs=rhs,
                            perf_mode=mybir.MatmulPerfMode.DoubleRow,
                            start=(tjd == 0),
                            stop=(tjd == NTT // 2 - 1),
                        )

                for tj in range(NTT):
                    tjs = slice(tj * P, (tj + 1) * P)
                    sp = pS.tile([P, 2 * QW], F32, tag="sp", name=f"sp_{p}_{q}_{tj}")
                    nc.tensor.matmul(
                        sp[:, 0:QW],
                        lhsT=kT[0:HD, tjs],
                        rhs=qT[0:HD, qs],
                        start=True,
                        stop=True,
                    )
                    nc.tensor.matmul(
                        sp[:, QW : 2 * QW],
                        lhsT=kT[HD:P, tjs],
                        rhs=qT[HD:P, qs],
                        start=True,
                        stop=True,
                    )
                    if tj % 2 == 0:
                        ptd = pP.tile(
                            [P, 4 * QW], FP8, tag="pt", name=f"pt_{p}_{q}_{tj}"
                        )
                        pts[tj // 2] = ptd
                    else:
                        ptd = pts[tj // 2]
                    nc.scalar.activation(
                        ptd[:, (tj % 2) * 2 * QW : (tj % 2 + 1) * 2 * QW],
                        sp,
                        mybir.ActivationFunctionType.Exp,
                    )
                    if tj >= 2 and tj % 2 == 0:
                        emit_pv(tj // 2 - 1)
                    for k in pulls.get(tj, []):
                        pull(k)
                    # emit previous unit's norm once this unit is underway
                    if tj == 1 and norm_pending is not None:
                        emit_norm(*norm_pending)
                        norm_pending = None
                emit_pv(NTT // 2 - 1)

                # evacuate po to SBUF so the PSUM banks free for the next
                # unit: both heads' O^T into one [128,512] pair tile, the two
                # denominator rows into [1,512] f32 tiles.
                po_sb = pPo.tile([P, QW], F32, tag="po_sb", name=f"posb_{p}_{q}")
                den_e = pPo.tile([1, QW], F32, tag="den_e", name=f"dene_{p}_{q}")
                den_o = pPo.tile([1, QW], F32, tag="den_o", name=f"deno_{p}_{q}")
                nc.vector.tensor_copy(den_e, poe[HD : HD + 1, :])
                nc.vector.tensor_copy(den_o, poo[HD : HD + 1, :])
                nc.vector.tensor_copy(po_sb[0:HD, :], poe[0:HD, :])
                nc.vector.tensor_copy(po_sb[HD:P, :], poo[0:HD, :])
                norm_pending = (p, q, po_sb, den_e, den_o)

        emit_norm(*norm_pending, pe_bcast=True)
        for mt in range(12, 16):
            pull(f"o{mt}")

    # bass.Bass's finalize skips Bacc's wait-splitting passes; walrus allows
    # at most 1 sync wait per instruction (2 for event semaphores), so run
    # just those two passes here.
    import bass_rust as _bass_rust

    _bass_rust.move_matmul_waits_to_ldweights(nc.m)
    _bass_rust.generate_event_semaphores(nc)
    return nc


def prepare_in_maps(inputs):
    q = np.asarray(inputs["query"], np.float32)
    ipw = np.asarray(inputs["in_proj_weight"], np.float32)
    ipb = np.asarray(inputs["in_proj_bias"], np.float32)
    out_w = np.asarray(inputs["out_w"], np.float32)
    k_a = np.asarray(inputs["k_a"], np.float32)
    k_b = np.asarray(inputs["k_b"], np.float32)
    v_a = np.asarray(inputs["v_a"], np.float32)
    v_b = np.asarray(inputs["v_b"], np.float32)
    qscale = 1.0 / math.sqrt(HD)
    sl = SCALE / R

    in_maps = []
    for c in range(NCORES):
        bb = c // 4
        s = (c % 4) * CD
        e = s + CD
        X = q[:, bb, :]

        xa = np.ascontiguousarray(X.T)

        wqk = np.empty((D, 2 * CD), np.float32)
        wqk[:, :CD] = ipw[s:e].T * qscale
        wqk[:, CD:] = ipw[D + s : D + e].T

        qkb = np.zeros((R + 1, 2 * CD), np.float32)
        qkb[:R, CD:] = k_b[:, s:e] * sl
        qkb[R, :CD] = ipb[s:e] * qscale
        qkb[R, CD:] = ipb[D + s : D + e]

        wv = np.zeros((D, WVW), np.float32)
        vbb = np.zeros((R + 1, WVW), np.float32)
        for j in range(HPC):
            wv[:, j * VW : j * VW + HD] = ipw[2 * D + s + j * HD : 2 * D + s + (j + 1) * HD].T
            vbb[:R, j * VW : j * VW + HD] = v_b[:, s + j * HD : s + (j + 1) * HD] * sl
            vbb[R, j * VW : j * VW + HD] = ipb[2 * D + s + j * HD : 2 * D + s + (j + 1) * HD]
            vbb[R, j * VW + HD] = 1.0

        abm = np.zeros((D, 3 * R), np.float32)
        abm[:, :R] = k_a.T
        abm[:, 2 * R :] = v_a.T

        wo = out_w[:, s:e].T

        in_maps.append(
            {
                "xa": xa.astype(BF16),
                "wqk": wqk.astype(BF16),
                "wv": wv.astype(BF16),
                "abm": abm.astype(BF16),
                "qkb": qkb.astype(BF16),
                "vbb": vbb.astype(BF16),
                "wo": wo.astype(BF16),
            }
        )
    return in_maps


def assemble_output(inputs, results):
    out_b = np.asarray(inputs["out_b"], np.float32)
    out = np.zeros((T, BSZ, D), np.float32)
    for c in range(NCORES):
        out[:, c // 4, :] += np.asarray(results[c]["out"], np.float32)
    out += out_b[None, None, :]
    return out


def kernel(**inputs):
    nc = build_nc()
    in_maps = prepare_in_maps(inputs)
    res = run_bass_kernel_spmd(nc, in_maps, core_ids=list(range(NCORES)))
    return assemble_output(inputs, res.results)


# revision 24
# speedup vs baseline: 1.2293x; 1.0660x over previous
"""LoRA MultiheadAttention on 8 NeuronCores (Bass/Tile), v3.

Sharding: 32 (batch, head) attention slices -> 4 heads x 1 batch per core.
Cores 0-3 take batch 0, cores 4-7 batch 1; core c handles heads
(c%4)*4 .. (c%4)*4+3, i.e. a contiguous 256-wide slice of the head dims.

Structure: one ACT-saturated attention stream with all projection work
interleaved as PE filler.

  - Units are (head-pair, ti-quarter).  Per tj tile the two heads' score
    matmuls (K=64) are emitted back-to-back with lhsT/rhs at partition
    bases 0 and 64, so they run CONCURRENTLY in different PE row strips
    (tile_position (0,0)/(64,0)) -- 2x on the score cost.
  - Both heads' scores land in one [128,1024] 2-bank PSUM tile, so each
    exp is a single [128,1024] ACT instruction.
  - Phases A (LoRA down-proj), B (Q/K proj), C (V proj) and the out-proj
    are pull-scheduled filler between attention steps.
  - Inputs arrive as a handful of large multi-k-tile DMAs (the Sync
    engine's ~0.65us per dma_start issue was gating the ramp); xa is
    split into column halves so the first units' data lands first.
  - Biases ride the K=17 LoRA matmuls (ones row in ak/av).  V gets its
    softmax-denominator ones-column the same way.
  - po is evacuated PSUM->SBUF right after each unit; denominator
    broadcast via DRAM round-trip (PE K=1 broadcast matmul for the last
    unit to cut the tail); one [128,512] reciprocal + one fused multiply
    per unit.
  - Output stored bf16 (host accumulates fp32).
"""

import sys

sys.path.insert(0, "/opt/trn_rl_repo")

import math
from contextlib import ExitStack

import ml_dtypes
import numpy as np

import concourse.bass as bass
import concourse.tile as tile
from concourse import mybir
from concourse.bass_utils import run_bass_kernel_spmd

BF16 = ml_dtypes.bfloat16
F32 = mybir.dt.float32
BF = mybir.dt.bfloat16
FP8 = mybir.dt.float8e4

T = 2048
D = 1024
H = 16
HD = 64
R = 16
BSZ = 2
SCALE = 16.0
NCORES = 8
HPC = 4  # heads per core
CD = HPC * HD  # 256 head dims per core
VW = HD + 1  # V block width per head (ones column appended)
P = 128
NKT = D // P  # 8 k-tiles
NTT = T // P  # 16 tj tiles
QW = 512  # ti quarter width
NQ = T // QW  # 4 quarters
WVW = HPC * VW  # 260


def build_nc():
    nc = bass.Bass()
    xa = nc.dram_tensor("xa", [D, T], BF, kind="ExternalInput")
    wqk = nc.dram_tensor("wqk", [D, 2 * CD], BF, kind="ExternalInput")
    wv = nc.dram_tensor("wv", [D, WVW], BF, kind="ExternalInput")
    abm = nc.dram_tensor("abm", [D, 3 * R], BF, kind="ExternalInput")
    qkb = nc.dram_tensor("qkb", [R + 1, 2 * CD], BF, kind="ExternalInput")
    vbb = nc.dram_tensor("vbb", [R + 1, WVW], BF, kind="ExternalInput")
    wo = nc.dram_tensor("wo", [CD, D], BF, kind="ExternalInput")
    out = nc.dram_tensor("out", [T, D], BF, kind="ExternalOutput")

    with tile.TileContext(nc) as tc, ExitStack() as ctx:
        singles = ctx.enter_context(tc.tile_pool(name="singles", bufs=1))

        # merged per-k-tile input tiles: [128, NKT*width], k-tile kt at
        # columns [kt*width, (kt+1)*width)
        xa_all = singles.tile([P, NKT * T], BF, tag="xa_all")
        wqk_all = singles.tile([P, NKT * 2 * CD], BF, tag="wqk_all")
        wv_all = singles.tile([P, NKT * WVW], BF, tag="wv_all")
        ab_all = singles.tile([P, NKT * 3 * R], BF, tag="ab_all")
        qkb_t = singles.tile([R + 1, 2 * CD], BF, tag="qkb")
        vbb_t = singles.tile([R + 1, WVW], BF, tag="vbb")
        wo_t = [singles.tile([P, D], BF, name=f"wo{i}", tag=f"wo{i}") for i in range(2)]
        ones64 = singles.tile([1, HD], F32, tag="ones64")
        nc.vector.memset(ones64, 1.0)

        def xs(kt, lo, hi):
            return xa_all[:, kt * T + lo : kt * T + hi]

        def wqks(kt, lo, hi):
            return wqk_all[:, kt * 2 * CD + lo : kt * 2 * CD + hi]

        def wvs(kt):
            return wv_all[:, kt * WVW : (kt + 1) * WVW]

        def abs_(kt):
            return ab_all[:, kt * 3 * R : (kt + 1) * 3 * R]

        # merged input DMAs: one start per tensor (xa in column halves),
        # ordered so the prologue's data lands first.
        def load_ktiled(dst_tile, dst_w, src, src_w, lo, hi):
            w = hi - lo
            dst = bass.AP(
                tensor=dst_tile.tensor,
                offset=dst_tile.offset + lo,
                ap=[list(dst_tile.ap[0]), [dst_w, NKT], [1, w]],
            )
            srcap = bass.AP(
                tensor=src[0:1, 0:1].tensor,
                offset=lo,
                ap=[[src_w, P], [P * src_w, NKT], [1, w]],
            )
            nc.sync.dma_start(out=dst, in_=srcap)

        def load_wqk_mgroup(g):
            # columns of m-tiles m=g and m=2+g for every k-tile (3-dim APs)
            for base in (g * P, CD + g * P):
                dst = bass.AP(
                    tensor=wqk_all.tensor,
                    offset=wqk_all.offset + base,
                    ap=[list(wqk_all.ap[0]), [2 * CD, NKT], [1, P]],
                )
                srcap = bass.AP(
                    tensor=wqk[0:1, 0:1].tensor,
                    offset=base,
                    ap=[[2 * CD, P], [P * 2 * CD, NKT], [1, P]],
                )
                nc.sync.dma_start(out=dst, in_=srcap)

        load_ktiled(xa_all, T, xa, T, 0, T // 4)
        load_ktiled(ab_all, 3 * R, abm, 3 * R, 0, 3 * R)
        nc.sync.dma_start(out=qkb_t, in_=qkb[:, :])
        load_wqk_mgroup(0)
        load_ktiled(xa_all, T, xa, T, T // 4, T // 2)
        load_ktiled(wv_all, WVW, wv, WVW, 0, WVW)
        nc.sync.dma_start(out=vbb_t, in_=vbb[:, :])
        load_wqk_mgroup(1)
        load_ktiled(xa_all, T, xa, T, T // 2, 3 * T // 4)
        load_ktiled(xa_all, T, xa, T, 3 * T // 4, T)
        for i in range(2):
            nc.sync.dma_start(out=wo_t[i], in_=wo[i * P : (i + 1) * P, :])

        # ak/av: LoRA down-projections with a trailing ones row (bias lane).
        # DVE accesses must start at partition 0/32/64/96: memset the whole
        # tile to 1.0 (covers the ones row); Phase A overwrites rows 0..15.
        ak_sb = singles.tile([R + 1, T], BF, tag="ak")
        av_sb = singles.tile([R + 1, T], BF, tag="av")
        nc.vector.memset(ak_sb, 1.0)
        nc.vector.memset(av_sb, 1.0)

        qk_sb = [singles.tile([P, T], BF, name=f"qk{i}", tag=f"qk{i}") for i in range(4)]
        VST = 272  # 16-aligned stride of one tj tile inside a v2 pair tile
        v2_sb = [singles.tile([P, 2 * VST], FP8, name=f"v2_{i}", tag=f"v2_{i}") for i in range(NTT // 2)]
        oT_sb = [singles.tile([P, T], BF, name=f"oT{i}", tag=f"oT{i}") for i in range(2)]

        # attention pools (long-lived, below proj on the pool stack)
        pS = ctx.enter_context(tc.tile_pool(name="pS", bufs=2, space="PSUM"))
        pO = ctx.enter_context(tc.tile_pool(name="pO", bufs=1, space="PSUM"))
        pP = ctx.enter_context(tc.tile_pool(name="pP", bufs=3))
        pN = ctx.enter_context(tc.tile_pool(name="pN", bufs=1))
        pPo = ctx.enter_context(tc.tile_pool(name="pPo", bufs=2))
        pD = ctx.enter_context(tc.tile_pool(name="pD", bufs=2, space="DRAM"))
        pOut = ctx.enter_context(tc.tile_pool(name="pOut", bufs=2))

        # proj pool: one rotating PSUM tag shared by phases A, B and C;
        # allocated last (stack top) and released once all A/B/C fillers
        # have been emitted so its banks recycle into the out-proj pool.
        proj = tc.alloc_tile_pool(name="proj", bufs=2, space="PSUM")

        def emit_a(ch):
            # [ak; av] chunk = [k_a; v_a] @ X   (rows 0..15 / 32..47)
            cs = slice(ch * 512, (ch + 1) * 512)
            pa = proj.tile([3 * R, 512], F32, tag="proj", name=f"pa{ch}")
            for kt in range(NKT):
                nc.tensor.matmul(
                    pa,
                    lhsT=abs_(kt),
                    rhs=xs(kt, ch * 512, (ch + 1) * 512),
                    start=(kt == 0),
                    stop=(kt == NKT - 1),
                )
            nc.vector.tensor_copy(ak_sb[0:R, cs], pa[0:R, :])
            nc.vector.tensor_copy(av_sb[0:R, cs], pa[2 * R : 3 * R, :])

        def emit_b(m, ch):
            # Q^T / K^T m-tile (m=0,1 -> Q pair m; m=2,3 -> K pair m-2), ti/tj
            # chunk ch.  8 k-tiles + 1 K=17 LoRA/bias matmul.
            cs = slice(ch * 512, (ch + 1) * 512)
            pq = proj.tile([P, 512], F32, tag="proj", name=f"pq_{m}_{ch}")
            for kt in range(NKT):
                nc.tensor.matmul(
                    pq,
                    lhsT=wqks(kt, m * P, (m + 1) * P),
                    rhs=xs(kt, ch * 512, (ch + 1) * 512),
                    start=(kt == 0),
                    stop=False,
                )
            nc.tensor.matmul(
                pq,
                lhsT=qkb_t[:, m * P : (m + 1) * P],
                rhs=ak_sb[:, cs],
                start=False,
                stop=True,
            )
            nc.vector.tensor_copy(qk_sb[m][:, cs], pq)

        def emit_c(mt):
            # V m-tile mt in natural (T, 4*65) layout; LoRA + bias + ones-col
            # via the K=17 matmul.
            pv = proj.tile([P, WVW], F32, tag="proj", name=f"pv_{mt}")
            for kt in range(NKT):
                nc.tensor.matmul(
                    pv,
                    lhsT=xs(kt, mt * P, (mt + 1) * P),
                    rhs=wvs(kt),
                    start=(kt == 0),
                    stop=False,
                )
            nc.tensor.matmul(
                pv,
                lhsT=av_sb[:, mt * P : (mt + 1) * P],
                rhs=vbb_t,
                start=False,
                stop=True,
            )
            nc.vector.tensor_copy(
                v2_sb[mt // 2][:, (mt % 2) * VST : (mt % 2) * VST + WVW], pv
            )

        # out-proj PSUM pool opened lazily (after proj retires) to stay
        # within 8 banks
        pE_box = {}

        def get_pE():
            if "pE" not in pE_box:
                pE_box["pE"] = ctx.enter_context(
                    tc.tile_pool(name="pE", bufs=2, space="PSUM")
                )
            return pE_box["pE"]

        ob_box = {}

        def emit_outproj_ch(mt, chh, on_act=False):
            pE = get_pE()
            ms = slice(mt * P, (mt + 1) * P)
            if mt not in ob_box:
                ob_box[mt] = pOut.tile([P, D], BF, tag="ob", name=f"ob_{mt}")
            ob = ob_box[mt]
            cs = slice(chh * 512, (chh + 1) * 512)
            po2 = pE.tile([P, 512], F32, tag="po2", name=f"po2_{mt}_{chh}")
            for kt2 in range(2):
                nc.tensor.matmul(
                    po2,
                    lhsT=oT_sb[kt2][:, ms],
                    rhs=wo_t[kt2][:, cs],
                    start=(kt2 == 0),
                    stop=(kt2 == 1),
                )
            if on_act:
                nc.scalar.copy(ob[:, cs], po2)
            else:
                nc.vector.tensor_copy(ob[:, cs], po2)
            nc.sync.dma_start(out=out[ms, cs], in_=ob[:, cs])

        fillers = {}
        for ch in range(4):
            fillers[f"a{ch}"] = (lambda ch=ch: emit_a(ch))
        for m in range(4):
            for ch in range(4):
                fillers[f"b{m}{ch}"] = (lambda m=m, ch=ch: emit_b(m, ch))
        for mt in range(NTT):
            fillers[f"c{mt}"] = (lambda mt=mt: emit_c(mt))
        for mt in range(NTT):
            for chh in range(2):
                fillers[f"o{mt}{'ab'[chh]}"] = (
                    lambda mt=mt, chh=chh: emit_outproj_ch(mt, chh)
                )
        fillers["release"] = proj.release

        def pull(key):
            f = fillers.pop(key, None)
            if f is not None:
                f()

        # prologue: minimal work for unit (pair 0, quarter 0)
        pull("b00")
        pull("a0")
        pull("b20")

        # filler pull schedule per (pair, quarter) unit: {step: [keys]}
        unit_pulls = {
            (0, 0): {
                -1: ["c0", "c1"],
                0: ["c2"], 1: ["c3"], 2: ["a1", "c4"], 3: ["b21", "c5"],
                4: ["c6"], 5: ["a2", "c7"], 6: ["b22", "c8"], 7: ["c9"],
                8: ["c10"], 9: ["a3", "c11"], 10: ["b23", "c12"], 11: ["c13"],
                12: ["c14"], 13: ["c15"], 14: ["b01"],
            },
            (0, 1): {0: ["b30"], 8: ["b02"], 12: ["b31"]},
            (0, 2): {0: ["b32"], 8: ["b03"]},
            (0, 3): {0: ["b33"], 12: ["b10"]},
            (1, 0): {4: ["b11"], 12: ["b12"]},
            (1, 1): {
                0: ["b13"], 1: ["release"],
                2: ["o0a"], 4: ["o0b"], 6: ["o1a"], 8: ["o1b"],
                10: ["o2a"], 12: ["o2b"], 14: ["o3a"], 15: ["o3b"],
            },
            (1, 2): {
                2: ["o4a"], 4: ["o4b"], 6: ["o5a"], 8: ["o5b"],
                10: ["o6a"], 12: ["o6b"], 14: ["o7a"], 15: ["o7b"],
            },
            (1, 3): {
                2: ["o8a"], 4: ["o8b"], 6: ["o9a"], 8: ["o9b"],
                10: ["o10a"], 12: ["o10b"], 14: ["o11a"], 15: ["o11b"],
            },
        }

        def emit_norm(p, q, po_sb, den_e, den_o, pe_bcast=False):
            # broadcast the two 512-wide denominators across 64 partitions
            # each, then one reciprocal and one fused [128,512] multiply for
            # both heads.  DRAM round-trip normally; K=1 PE broadcast matmul
            # for the last unit (cuts two DMA hops from the tail).
            qs = slice(q * QW, (q + 1) * QW)
            if pe_bcast:
                pE = get_pE()
                pbc = pE.tile([P, QW], F32, tag="po2", name=f"pbc_{p}_{q}")
                nc.tensor.matmul(
                    pbc[0:HD, :], lhsT=ones64, rhs=den_e, start=True, stop=True
                )
                nc.tensor.matmul(
                    pbc[HD:P, :], lhsT=ones64, rhs=den_o, start=True, stop=True
                )
                d128 = pbc
            else:
                dre = pD.tile([1, QW], F32, tag="dre", name=f"dre_{p}_{q}")
                dro = pD.tile([1, QW], F32, tag="dro", name=f"dro_{p}_{q}")
                nc.sync.dma_start(out=dre, in_=den_e)
                nc.sync.dma_start(out=dro, in_=den_o)
                d128 = pN.tile([P, QW], F32, tag="d128", name=f"d128_{p}_{q}")
                nc.sync.dma_start(
                    out=d128[0:HD, :],
                    in_=bass.AP(
                        tensor=dre.tensor, offset=dre.offset, ap=[[0, HD], [1, QW]]
                    ),
                )
                nc.sync.dma_start(
                    out=d128[HD:P, :],
                    in_=bass.AP(
                        tensor=dro.tensor, offset=dro.offset, ap=[[0, HD], [1, QW]]
                    ),
                )
            rec = pN.tile([P, QW], F32, tag="rec", name=f"rec_{p}_{q}")
            for cc in range(4):
                ccs = slice(cc * P, (cc + 1) * P)
                ocs = slice(q * QW + cc * P, q * QW + (cc + 1) * P)
                nc.vector.reciprocal(out=rec[:, ccs], in_=d128[:, ccs])
                nc.vector.tensor_mul(oT_sb[p][:, ocs], po_sb[:, ccs], rec[:, ccs])

        norm_pending = None
        for p in range(2):
            qT = qk_sb[p]
            kT = qk_sb[2 + p]
            for q in range(NQ):
                qs = slice(q * QW, (q + 1) * QW)
                pulls = unit_pulls[(p, q)]
                for k in pulls.get(-1, []):
                    pull(k)
                poe = pO.tile([VW, QW], F32, tag="poe", name=f"poe_{p}_{q}")
                poo = pO.tile([VW, QW], F32, tag="poo", name=f"poo_{p}_{q}")
                pts = {}

                def emit_pv(tjd, poe=poe, poo=poo, p=p):
                    # fp8 DoubleRow: one matmul contracts both tj tiles of a
                    # pair (virtual K=256); lhsT/rhs carry the pair in dim 1.
                    ptd = pts.pop(tjd)
                    v2 = v2_sb[tjd]
                    for hloc, po_ in ((2 * p, poe), (2 * p + 1, poo)):
                        lhsT = bass.AP(
                            tensor=v2.tensor,
                            offset=v2.offset + hloc * VW,
                            ap=[list(v2.ap[0]), [VST, 2], [1, VW]],
                        )
                        rhs = bass.AP(
                            tensor=ptd.tensor,
                            offset=ptd.offset + (hloc - 2 * p) * QW,
                            ap=[list(ptd.ap[0]), [2 * QW, 2], [1, QW]],
                        )
                        nc.tensor.matmul(
                            po_,
                            lhsT=lhsT,
                            rhs=rhs,
                            perf_mode=mybir.MatmulPerfMode.DoubleRow,
                            start=(tjd == 0),
                            stop=(tjd == NTT // 2 - 1),
                        )

                for tj in range(NTT):
                    tjs = slice(tj * P, (tj + 1) * P)
                    sp = pS.tile([P, 2 * QW], F32, tag="sp", name=f"sp_{p}_{q}_{tj}")
                    nc.tensor.matmul(
                        sp[:, 0:QW],
                        lhsT=kT[0:HD, tjs],
                        rhs=qT[0:HD, qs],
                        start=True,
                        stop=True,
                    )
                    nc.tensor.matmul(
                        sp[:, QW : 2 * QW],
                        lhsT=kT[HD:P, tjs],
                        rhs=qT[HD:P, qs],
                        start=True,
                        stop=True,
                    )
                    if tj % 2 == 0:
                        ptd = pP.tile(
                            [P, 4 * QW], FP8, tag="pt", name=f"pt_{p}_{q}_{tj}"
                        )
                        pts[tj // 2] = ptd
                    else:
                        ptd = pts[tj // 2]
                    nc.scalar.activation(
                        ptd[:, (tj % 2) * 2 * QW : (tj % 2 + 1) * 2 * QW],
                        sp,
                        mybir.ActivationFunctionType.Exp,
                    )
                    if tj >= 2 and tj % 2 == 0:
                        emit_pv(tj // 2 - 1)
                    for k in pulls.get(tj, []):
                        pull(k)
                    # emit previous unit's norm once this unit is underway
                    if tj == 1 and norm_pending is not None:
                        emit_norm(*norm_pending)
                        norm_pending = None
                emit_pv(NTT // 2 - 1)

                # evacuate po to SBUF so the PSUM banks free for the next
                # unit: both heads' O^T into one [128,512] pair tile, the two
                # denominator rows into [1,512] f32 tiles.
                po_sb = pPo.tile([P, QW], F32, tag="po_sb", name=f"posb_{p}_{q}")
                den_e = pPo.tile([1, QW], F32, tag="den_e", name=f"dene_{p}_{q}")
                den_o = pPo.tile([1, QW], F32, tag="den_o", name=f"deno_{p}_{q}")
                nc.vector.tensor_copy(den_e, poe[HD : HD + 1, :])
                nc.vector.tensor_copy(den_o, poo[HD : HD + 1, :])
                nc.vector.tensor_copy(po_sb[0:HD, :], poe[0:HD, :])
                nc.vector.tensor_copy(po_sb[HD:P, :], poo[0:HD, :])
                norm_pending = (p, q, po_sb, den_e, den_o)

        emit_norm(*norm_pending, pe_bcast=True)
        for mt in range(12, 16):
            for chh in range(2):
                fillers.pop(f"o{mt}{'ab'[chh]}")
                emit_outproj_ch(mt, chh, on_act=True)

    # bass.Bass's finalize skips Bacc's wait-splitting passes; walrus allows
    # at most 1 sync wait per instruction (2 for event semaphores), so run
    # just those two passes here.
    import bass_rust as _bass_rust

    _bass_rust.move_matmul_waits_to_ldweights(nc.m)
    _bass_rust.generate_event_semaphores(nc)
    return nc


def prepare_in_maps(inputs):
    q = np.asarray(inputs["query"], np.float32)
    ipw = np.asarray(inputs["in_proj_weight"], np.float32)
    ipb = np.asarray(inputs["in_proj_bias"], np.float32)
    out_w = np.asarray(inputs["out_w"], np.float32)
    k_a = np.asarray(inputs["k_a"], np.float32)
    k_b = np.asarray(inputs["k_b"], np.float32)
    v_a = np.asarray(inputs["v_a"], np.float32)
    v_b = np.asarray(inputs["v_b"], np.float32)
    qscale = 1.0 / math.sqrt(HD)
    sl = SCALE / R

    in_maps = []
    for c in range(NCORES):
        bb = c // 4
        s = (c % 4) * CD
        e = s + CD
        X = q[:, bb, :]

        xa = np.ascontiguousarray(X.T)

        wqk = np.empty((D, 2 * CD), np.float32)
        wqk[:, :CD] = ipw[s:e].T * qscale
        wqk[:, CD:] = ipw[D + s : D + e].T

        qkb = np.zeros((R + 1, 2 * CD), np.float32)
        qkb[:R, CD:] = k_b[:, s:e] * sl
        qkb[R, :CD] = ipb[s:e] * qscale
        qkb[R, CD:] = ipb[D + s : D + e]

        wv = np.zeros((D, WVW), np.float32)
        vbb = np.zeros((R + 1, WVW), np.float32)
        for j in range(HPC):
            wv[:, j * VW : j * VW + HD] = ipw[2 * D + s + j * HD : 2 * D + s + (j + 1) * HD].T
            vbb[:R, j * VW : j * VW + HD] = v_b[:, s + j * HD : s + (j + 1) * HD] * sl
            vbb[R, j * VW : j * VW + HD] = ipb[2 * D + s + j * HD : 2 * D + s + (j + 1) * HD]
            vbb[R, j * VW + HD] = 1.0

        abm = np.zeros((D, 3 * R), np.float32)
        abm[:, :R] = k_a.T
        abm[:, 2 * R :] = v_a.T

        wo = out_w[:, s:e].T

        in_maps.append(
            {
                "xa": xa.astype(BF16),
                "wqk": wqk.astype(BF16),
                "wv": wv.astype(BF16),
                "abm": abm.astype(BF16),
                "qkb": qkb.astype(BF16),
                "vbb": vbb.astype(BF16),
                "wo": wo.astype(BF16),
            }
        )
    return in_maps


def assemble_output(inputs, results):
    out_b = np.asarray(inputs["out_b"], np.float32)
    out = np.zeros((T, BSZ, D), np.float32)
    for c in range(NCORES):
        out[:, c // 4, :] += np.asarray(results[c]["out"], np.float32)
    out += out_b[None, None, :]
    return out


def kernel(**inputs):
    nc = build_nc()
    in_maps = prepare_in_maps(inputs)
    res = run_bass_kernel_spmd(nc, in_maps, core_ids=list(range(NCORES)))
    return assemble_output(inputs, res.results)
